# revision 11
# baseline (speedup 1.0000x reference)
"""Trainium2 Bass kernel for nn_BinaryDiceLoss (sum of per-pixel BCE).

loss = sum_{b,h,w} mean_c[-(t*log(p) + (1-t)*log(1-p))], shapes [32,1,1024,1024] f32.

Sharding: data-parallel over batch - 4 images (4.19M elements) per NeuronCore
on 8 cores.

v4 design ("canonical half-range, split-diag"): host canonicalizes each
element to p' = max(p, 1-p), t' = (p >= 0.5 ? t : 1-t) -- bce is symmetric
under (p,t) -> (1-p,1-t) -- then uniformly quantizes p' to u8:
c = floor(256*p') in [128,255], p_hat = (c+0.5)/256 in [0.5, 1).

p_hat spans ONE fp16 binade, so bits_i16(fp16(p_hat)) = 13316 + 8c EXACTLY
and the fp16 log-bit-hack ln(x) ~= A*bits + B (A = ln2/1024, mean-zero error
on the 128-point mantissa grid) makes log(p') AFFINE IN THE RAW CODE:
    v := -log(p') = K - 8A*c.
Identity: bce = t'*(log1mp' - logp') - log1mp', so with u = lg + v:
  ACT:  lg = Ln((255.5-c)/256) = log(1-p')  [1x from u8, accum_out =>
        sum(log1mp'), the only reduction needed], strided out into uv block 0
  DVE:  v = K - 8A*c (ts u8->bf16, the ONLY DVE pass), into uv block 1
  PE:   per 128-col chunk, ONE matmul with moving = uv[:, chunk, :, :]
        (FD=256: [lg_chunk | v_chunk]):  psum[128,256] += t'_c.T @ [lg|v]
        diag slots [i,i] and [i,128+i] accumulate sum(t'*lg) and sum(t'*v);
        the add lg+v happens IN PSUM, not on DVE.
Host: total = (diag_a + diag_b) - sum(asums)  (f64, f32 out).

1B predict + 1B target = 2B/elem DMA.  Expected rel err ~1.2e-3 (u8
quantization bias; all other errors mean-zero), vs the 2e-2 gate.
"""

import math

import numpy as np

_N_CORES = 8
_P = 128
_PER_CORE = 32 * 1024 * 1024 // _N_CORES // _P  # 32768 columns of 128
_SEGS = [1024, 1024, 2048] + [8192] * 3 + [2048, 1024, 1024]
assert sum(_SEGS) == _PER_CORE

_LN2 = math.log(2.0)
_A = _LN2 / 1024.0
_B = -15.0 * _LN2 + (1.5 * _LN2 - 1.0)
_K = -(13316.0 * _A + _B)  # v = K - 8A*c = -log(p') under the bit-hack

_CACHED_NC = None
LAST_RESULTS = None  # BassKernelResults of the most recent run (for harnesses)


def _seg_classes():
    """Group segments by size: {fl: count}, preserving per-class order."""
    counts = {}
    for fl in _SEGS:
        counts[fl] = counts.get(fl, 0) + 1
    return counts


def _build():
    import concourse.bacc as bacc
    import concourse.tile as tile
    from concourse import mybir

    f32 = mybir.dt.float32
    bf16 = mybir.dt.bfloat16
    fp16 = mybir.dt.float16
    u8 = mybir.dt.uint8
    fp8 = mybir.dt.float8e4
    Alu = mybir.AluOpType
    p = _P

    nc = bacc.Bacc(
        "TRN2",
        target_bir_lowering=False,
        debug=False,
        enable_asserts=False,
        num_devices=_N_CORES,
    )
    counts = _seg_classes()
    pred = {
        fl: nc.dram_tensor(f"p{fl}", [n, p, fl], u8, kind="ExternalInput").ap()
        for fl, n in counts.items()
    }
    targ = {
        fl: nc.dram_tensor(f"t{fl}", [n, p, fl], fp8, kind="ExternalInput").ap()
        for fl, n in counts.items()
    }
    nseg = len(_SEGS)
    out_b = nc.dram_tensor("out_b", [p, nseg], f32, kind="ExternalOutput").ap()
    out_d = nc.dram_tensor("out_d", [p, 2 * p], f32, kind="ExternalOutput").ap()

    io_bufs = {1024: 4, 2048: 2, 8192: 3}
    wk_bufs = {1024: 2, 2048: 2, 8192: 2}

    with tile.TileContext(nc) as tc:
        with (
            tc.tile_pool(name="cin", bufs=1) as cin,
            tc.tile_pool(name="tin", bufs=1) as tin,
            tc.tile_pool(name="uv", bufs=1) as uvp,
            tc.tile_pool(name="accs", bufs=1) as accs,
            tc.tile_pool(name="ps", bufs=1, space="PSUM") as ps,
        ):
            asums = accs.tile([p, nseg], f32, tag="asums")
            qb = accs.tile([p, 1], f32, tag="qb")
            # memset on gpsimd (ready ~immediately) so the warmup Ln -- and
            # with it the ~1.3us ACT_TABLE_LOAD -- issues as early as
            # possible instead of waiting on the Vector engine's preamble.
            nc.gpsimd.memset(qb, 255.5 / 256.0)
            warm = accs.tile([p, 1], fp16, tag="warm")
            nc.scalar.activation(
                out=warm, in_=qb, func=mybir.ActivationFunctionType.Ln,
                bias=qb[:, :], scale=0.0,
            )
            psum = ps.tile([p, 2 * p], f32, tag="psum")

            cls_idx = {fl: 0 for fl in counts}
            cts = {}

            def fetch_c(s):
                fl = _SEGS[s]
                i = cls_idx[fl]
                ct = cin.tile([p, fl], u8, tag=f"c{fl}", bufs=io_bufs[fl])
                nc.sync.dma_start(out=ct, in_=pred[fl][i, :, :])
                cts[s] = (ct, fl, i)
                cls_idx[fl] = i + 1

            fetch_c(0)
            fetch_c(1)
            for s in range(nseg):
                if s + 2 < nseg:
                    fetch_c(s + 2)
                ct, fl, i = cts.pop(s)
                wt = tin.tile([p, fl], fp8, tag=f"t{fl}", bufs=io_bufs[fl])
                nc.sync.dma_start(out=wt, in_=targ[fl][i, :, :])
                nch = fl // p
                # uv[:, c, 0, :] = lg chunk, uv[:, c, 1, :] = v chunk
                uv = uvp.tile([p, nch, 2, p], bf16, tag=f"uv{fl}",
                              bufs=wk_bufs[fl])
                # lg = Ln((255.5-c)/256) = log(1-p'); accum -> sum(log1mp')
                nc.scalar.activation(
                    out=uv[:, :, 0, :], in_=ct,
                    func=mybir.ActivationFunctionType.Ln,
                    bias=qb[:, :], scale=-1.0 / 256.0,
                    accum_out=asums[:, s:s + 1],
                )
                # v = K - 8A*c = -log(p') (bit-hack; affine on this binade)
                # split >4096 free-dims in half: DVE 2x-mode degrades past
                # 4096 free elements per op
                if fl > 4096:
                    h = nch // 2
                    nc.vector.tensor_scalar(uv[:, :h, 1, :], ct[:, :fl // 2],
                                            -8.0 * _A, _K, Alu.mult, Alu.add)
                    nc.vector.tensor_scalar(uv[:, h:, 1, :], ct[:, fl // 2:],
                                            -8.0 * _A, _K, Alu.mult, Alu.add)
                else:
                    nc.vector.tensor_scalar(uv[:, :, 1, :], ct, -8.0 * _A, _K,
                                            Alu.mult, Alu.add)
                for c in range(nch):
                    sl = slice(c * p, (c + 1) * p)
                    nc.tensor.matmul(
                        psum[:, :],
                        wt[:, sl],
                        uv[:, c, :, :],
                        start=(s == 0 and c == 0),
                        stop=(s == nseg - 1 and c == nch - 1),
                    )
            nc.sync.dma_start(out=out_b, in_=asums, single_packet=True)
            dcopy = accs.tile([p, 2 * p], f32, tag="dcopy")
            nc.vector.tensor_copy(dcopy, psum)
            nc.sync.dma_start(out=out_d, in_=dcopy, single_packet=True)

    nc.compile()
    return nc


def kernel(predict: np.ndarray, target: np.ndarray, _trace: bool = False) -> np.ndarray:
    global _CACHED_NC, LAST_RESULTS
    from concourse.bass_utils import run_bass_kernel_spmd
    import ml_dtypes

    predict = np.asarray(predict)
    target = np.asarray(target)
    assert predict.shape == (32, 1, 1024, 1024) and predict.dtype == np.float32
    assert target.shape == (32, 1, 1024, 1024) and target.dtype == np.float32

    if _CACHED_NC is None:
        _CACHED_NC = _build()
    nc = _CACHED_NC

    counts = _seg_classes()
    pr = np.ascontiguousarray(predict).reshape(_N_CORES, _PER_CORE * _P)
    tg = np.ascontiguousarray(target).reshape(_N_CORES, _PER_CORE * _P)
    c0 = (pr * 256.0).astype(np.uint8)
    flip = c0 < 128
    cc = np.where(flip, 255 - c0, c0)                      # c' in [128,255]
    tt = np.where(flip, 1.0 - tg, tg).astype(np.float32)   # t'
    t8 = tt.astype(ml_dtypes.float8_e4m3)

    # carve the flat per-core stream into per-size-class stacks, in order
    in_maps = [dict() for _ in range(_N_CORES)]
    off = 0
    cls_i = {fl: 0 for fl in counts}
    segs_np = {
        fl: (np.empty((_N_CORES, n, _P, fl), np.uint8),
             np.empty((_N_CORES, n, _P, fl), ml_dtypes.float8_e4m3))
        for fl, n in counts.items()
    }
    for fl in _SEGS:
        n = _P * fl
        i = cls_i[fl]
        segs_np[fl][0][:, i] = cc[:, off:off + n].reshape(_N_CORES, _P, fl)
        segs_np[fl][1][:, i] = t8[:, off:off + n].reshape(_N_CORES, _P, fl)
        cls_i[fl] = i + 1
        off += n
    for c in range(_N_CORES):
        for fl in counts:
            in_maps[c][f"p{fl}"] = segs_np[fl][0][c]
            in_maps[c][f"t{fl}"] = segs_np[fl][1][c]

    res = run_bass_kernel_spmd(
        nc, in_maps, core_ids=list(range(_N_CORES)), trace=_trace,
    )
    LAST_RESULTS = res
    # psum[:, :128] diag = sum(t'*lg); psum[:, 128:] diag = sum(t'*v);
    # asums = sum(log1mp').  total = diag_a + diag_b - sum(asums).
    total = 0.0
    for c in range(_N_CORES):
        d = np.asarray(res.results[c]["out_d"], dtype=np.float64)
        total += float(np.trace(d[:, :_P])) + float(np.trace(d[:, _P:]))
        total -= float(np.sum(res.results[c]["out_b"], dtype=np.float64))
    return np.array(total, dtype=np.float32)


# revision 15
# speedup vs baseline: 1.1074x; 1.1074x over previous
"""Trainium2 Bass kernel for nn_BinaryDiceLoss (sum of per-pixel BCE).

loss = sum_{b,h,w} mean_c[-(t*log(p) + (1-t)*log(1-p))], shapes [32,1,1024,1024] f32.

Sharding: data-parallel over batch - 4 images (4.19M elements) per NeuronCore
on 8 cores.

v4 design ("canonical half-range, split-diag"): host canonicalizes each
element to p' = max(p, 1-p), t' = (p >= 0.5 ? t : 1-t) -- bce is symmetric
under (p,t) -> (1-p,1-t) -- then uniformly quantizes p' to u8:
c = floor(256*p') in [128,255], p_hat = (c+0.5)/256 in [0.5, 1).

p_hat spans ONE fp16 binade, so bits_i16(fp16(p_hat)) = 13316 + 8c EXACTLY
and the fp16 log-bit-hack ln(x) ~= A*bits + B (A = ln2/1024, mean-zero error
on the 128-point mantissa grid) makes log(p') AFFINE IN THE RAW CODE:
    v := -log(p') = K - 8A*c.
Identity: bce = t'*(log1mp' - logp') - log1mp', so with u = lg + v:
  ACT:  lg = Ln((255.5-c)/256) = log(1-p')  [1x from u8, accum_out =>
        sum(log1mp'), the only reduction needed], strided out into uv block 0
  DVE:  v = K - 8A*c (ts u8->bf16, the ONLY DVE pass), into uv block 1
  PE:   per 128-col chunk, ONE matmul with moving = uv[:, chunk, :, :]
        (FD=256: [lg_chunk | v_chunk]):  psum[128,256] += t'_c.T @ [lg|v]
        diag slots [i,i] and [i,128+i] accumulate sum(t'*lg) and sum(t'*v);
        the add lg+v happens IN PSUM, not on DVE.
Host: total = (diag_a + diag_b) - sum(asums)  (f64, f32 out).

1B predict + 1B target = 2B/elem DMA.  Expected rel err ~1.2e-3 (u8
quantization bias; all other errors mean-zero), vs the 2e-2 gate.
"""

import math

import numpy as np

_N_CORES = 8
_P = 128
_PER_CORE = 32 * 1024 * 1024 // _N_CORES // _P  # 32768 columns of 128
_SEGS = [1024, 1024, 2048] + [4096] * 6 + [2048, 1024, 1024]
assert sum(_SEGS) == _PER_CORE
# segments whose lg+v add runs on DVE (moving FD=128) instead of in PSUM
# (moving FD=256): trades idle DVE capacity for Tensor-array cycles.
_DVE_ADD = {3, 4, 5}

_LN2 = math.log(2.0)
_A = _LN2 / 1024.0
_B = -15.0 * _LN2 + (1.5 * _LN2 - 1.0)
_K = -(13316.0 * _A + _B)  # v = K - 8A*c = -log(p') under the bit-hack

_CACHED_NC = None
LAST_RESULTS = None  # BassKernelResults of the most recent run (for harnesses)


def _seg_classes():
    """Group segments by size: {fl: count}, preserving per-class order."""
    counts = {}
    for fl in _SEGS:
        counts[fl] = counts.get(fl, 0) + 1
    return counts


def _build():
    import concourse.bacc as bacc
    import concourse.tile as tile
    from concourse import mybir

    f32 = mybir.dt.float32
    bf16 = mybir.dt.bfloat16
    fp16 = mybir.dt.float16
    u8 = mybir.dt.uint8
    fp8 = mybir.dt.float8e4
    Alu = mybir.AluOpType
    p = _P

    nc = bacc.Bacc(
        "TRN2",
        target_bir_lowering=False,
        debug=False,
        enable_asserts=False,
        num_devices=_N_CORES,
    )
    counts = _seg_classes()
    pred = {
        fl: nc.dram_tensor(f"p{fl}", [n, p, fl], u8, kind="ExternalInput").ap()
        for fl, n in counts.items()
    }
    targ = {
        fl: nc.dram_tensor(f"t{fl}", [n, p, fl], fp8, kind="ExternalInput").ap()
        for fl, n in counts.items()
    }
    nseg = len(_SEGS)
    out_b = nc.dram_tensor("out_b", [p, nseg], f32, kind="ExternalOutput").ap()
    out_d = nc.dram_tensor("out_d", [p, 2 * p], f32, kind="ExternalOutput").ap()

    io_bufs = {1024: 4, 2048: 3, 4096: 5}
    wk_bufs = {1024: 2, 2048: 2, 4096: 4}

    with tile.TileContext(nc) as tc:
        with (
            tc.tile_pool(name="cin", bufs=1) as cin,
            tc.tile_pool(name="tin", bufs=1) as tin,
            tc.tile_pool(name="uv", bufs=1) as uvp,
            tc.tile_pool(name="accs", bufs=1) as accs,
            tc.tile_pool(name="ps", bufs=1, space="PSUM") as ps,
        ):
            asums = accs.tile([p, nseg], f32, tag="asums")
            qb = accs.tile([p, 1], f32, tag="qb")
            # memset on gpsimd (ready ~immediately) so the warmup Ln -- and
            # with it the ~1.3us ACT_TABLE_LOAD -- issues as early as
            # possible instead of waiting on the Vector engine's preamble.
            nc.gpsimd.memset(qb, 255.5 / 256.0)
            warm = accs.tile([p, 1], fp16, tag="warm")
            nc.scalar.activation(
                out=warm, in_=qb, func=mybir.ActivationFunctionType.Ln,
                bias=qb[:, :], scale=0.0,
            )
            psum = ps.tile([p, 2 * p], f32, tag="psum")

            cls_idx = {fl: 0 for fl in counts}
            cts = {}

            def fetch_c(s):
                fl = _SEGS[s]
                i = cls_idx[fl]
                ct = cin.tile([p, fl], u8, tag=f"c{fl}", bufs=io_bufs[fl])
                nc.sync.dma_start(out=ct, in_=pred[fl][i, :, :])
                cts[s] = (ct, fl, i)
                cls_idx[fl] = i + 1

            fetch_c(0)
            fetch_c(1)
            for s in range(nseg):
                if s + 2 < nseg:
                    fetch_c(s + 2)
                ct, fl, i = cts.pop(s)
                wt = tin.tile([p, fl], fp8, tag=f"t{fl}", bufs=io_bufs[fl])
                nc.sync.dma_start(out=wt, in_=targ[fl][i, :, :])
                nch = fl // p
                # uv[:, c, 0, :] = lg chunk, uv[:, c, 1, :] = v chunk
                uv = uvp.tile([p, nch, 2, p], bf16, tag=f"uv{fl}",
                              bufs=wk_bufs[fl])
                # lg = Ln((255.5-c)/256) = log(1-p'); accum -> sum(log1mp')
                nc.scalar.activation(
                    out=uv[:, :, 0, :], in_=ct,
                    func=mybir.ActivationFunctionType.Ln,
                    bias=qb[:, :], scale=-1.0 / 256.0,
                    accum_out=asums[:, s:s + 1],
                )
                # v = K - 8A*c = -log(p') (bit-hack; affine on this binade)
                nc.vector.tensor_scalar(uv[:, :, 1, :], ct, -8.0 * _A, _K,
                                        Alu.mult, Alu.add)
                if s in _DVE_ADD:
                    # lg += v on DVE; matmul moving is just the lg block
                    nc.vector.tensor_add(uv[:, :, 0, :], uv[:, :, 0, :],
                                         uv[:, :, 1, :])
                for c in range(nch):
                    sl = slice(c * p, (c + 1) * p)
                    if s in _DVE_ADD:
                        out_ap, mov = psum[:, 0:p], uv[:, c, 0, :]
                    else:
                        out_ap, mov = psum[:, :], uv[:, c, :, :]
                    nc.tensor.matmul(
                        out_ap,
                        wt[:, sl],
                        mov,
                        start=(s == 0 and c == 0),
                        stop=(s == nseg - 1 and c == nch - 1),
                    )
            nc.sync.dma_start(out=out_b, in_=asums, single_packet=True)
            dcopy = accs.tile([p, 2 * p], f32, tag="dcopy")
            nc.vector.tensor_copy(dcopy, psum)
            nc.sync.dma_start(out=out_d, in_=dcopy, single_packet=True)

    nc.compile()
    return nc


def kernel(predict: np.ndarray, target: np.ndarray, _trace: bool = False) -> np.ndarray:
    global _CACHED_NC, LAST_RESULTS
    from concourse.bass_utils import run_bass_kernel_spmd
    import ml_dtypes

    predict = np.asarray(predict)
    target = np.asarray(target)
    assert predict.shape == (32, 1, 1024, 1024) and predict.dtype == np.float32
    assert target.shape == (32, 1, 1024, 1024) and target.dtype == np.float32

    if _CACHED_NC is None:
        _CACHED_NC = _build()
    nc = _CACHED_NC

    counts = _seg_classes()
    pr = np.ascontiguousarray(predict).reshape(_N_CORES, _PER_CORE * _P)
    tg = np.ascontiguousarray(target).reshape(_N_CORES, _PER_CORE * _P)
    c0 = (pr * 256.0).astype(np.uint8)
    flip = c0 < 128
    cc = np.where(flip, 255 - c0, c0)                      # c' in [128,255]
    tt = np.where(flip, 1.0 - tg, tg).astype(np.float32)   # t'
    t8 = tt.astype(ml_dtypes.float8_e4m3)

    # carve the flat per-core stream into per-size-class stacks, in order
    in_maps = [dict() for _ in range(_N_CORES)]
    off = 0
    cls_i = {fl: 0 for fl in counts}
    segs_np = {
        fl: (np.empty((_N_CORES, n, _P, fl), np.uint8),
             np.empty((_N_CORES, n, _P, fl), ml_dtypes.float8_e4m3))
        for fl, n in counts.items()
    }
    for fl in _SEGS:
        n = _P * fl
        i = cls_i[fl]
        segs_np[fl][0][:, i] = cc[:, off:off + n].reshape(_N_CORES, _P, fl)
        segs_np[fl][1][:, i] = t8[:, off:off + n].reshape(_N_CORES, _P, fl)
        cls_i[fl] = i + 1
        off += n
    for c in range(_N_CORES):
        for fl in counts:
            in_maps[c][f"p{fl}"] = segs_np[fl][0][c]
            in_maps[c][f"t{fl}"] = segs_np[fl][1][c]

    res = run_bass_kernel_spmd(
        nc, in_maps, core_ids=list(range(_N_CORES)), trace=_trace,
    )
    LAST_RESULTS = res
    # psum[:, :128] diag = sum(t'*lg); psum[:, 128:] diag = sum(t'*v);
    # asums = sum(log1mp').  total = diag_a + diag_b - sum(asums).
    total = 0.0
    for c in range(_N_CORES):
        d = np.asarray(res.results[c]["out_d"], dtype=np.float64)
        total += float(np.trace(d[:, :_P])) + float(np.trace(d[:, _P:]))
        total -= float(np.sum(res.results[c]["out_b"], dtype=np.float64))
    return np.array(total, dtype=np.float32)
